# revision 6
# baseline (speedup 1.0000x reference)
"""Multi-head attention (B=4, N=2048, C=1024, H=16) on 8 trn2 NeuronCores.

Sharding: data-parallel over batch (4) x tensor-parallel over heads (2).
Core c handles batch c//2, heads [8*(c%2), 8*(c%2)+8). Host sums core pairs
and adds the projection bias.

v1 rework vs baseline:
- bf16 inputs everywhere (x, wq/wk/wv/wp, mask); out returned as bf16 and
  upcast+summed on host. Halves DMA traffic and SBUF footprint.
- Phase 1 (QKV) PSUM evacuations moved to ScalarE (idle in phase 1), keeping
  VectorE free; TensorE runs the projections back-to-back.
- Phase 2 restructured around the ScalarE exp floor: two rotating score PSUM
  tiles ([128,1024] fp32 = 2 banks each) so ACTIVATE runs back-to-back;
  score matmul pairs (K=64, heads 2hb/2hb+1) emitted interleaved so they run
  concurrently in different PE row groups.
- Softmax normalization fused into phase 2: the denominator row (ones column
  in vaug) is reciprocal'd on DVE at partition 64, broadcast across 64
  partitions by GpSimd (partition_broadcast), and multiplied in-place into
  the evacuated bf16 attn output. No tail normalization sweep.
- Mask DMA'd on the gpsimd queue in parallel with x/w on the sync queue.
"""

import os
import sys

for p in ("/opt/trn_rl_repo", "/root/.axon_site/_ro/trn_rl_repo"):
    if os.path.isdir(p) and p not in sys.path:
        sys.path.insert(0, p)

import ml_dtypes
import numpy as np

import concourse.bacc as bacc
import concourse.tile as tile
from concourse import mybir
from concourse.bass_utils import run_bass_kernel_spmd

FP = mybir.dt.float32
FR = mybir.dt.float32r
BF = mybir.dt.bfloat16
EXP = mybir.ActivationFunctionType.Exp

DIM = 1024
NUM_HEADS = 16
HEAD_DIM = 64
SCALE = HEAD_DIM ** -0.5
B, N = 4, 2048
NCORES = 8


def build_attention(n=N, c=DIM, cp=DIM // 2, hd=HEAD_DIM, scale=SCALE):
    """Emit the per-core program. All cores run the same code (SPMD)."""
    hpc = cp // hd          # heads on this core (8)
    CB = c // 128           # contraction blocks for QKV (8)
    MB = cp // 128          # c' blocks (q/k transposed layout) (4)
    NB = n // 128           # token/key blocks (16)
    QC = n // 512           # 512-col chunks over tokens (4)
    QW = 1024               # phase-2 q chunk width
    QH = n // QW            # phase-2 outer q chunks (2)
    hd1 = hd + 1            # v augmented with ones column -> denominator

    nc = bacc.Bacc("TRN2", target_bir_lowering=False, debug=False)

    xT = nc.dram_tensor("xT", [c, n], BF, kind="ExternalInput").ap()
    wqT = nc.dram_tensor("wqT", [c, cp], BF, kind="ExternalInput").ap()
    wkT = nc.dram_tensor("wkT", [c, cp], BF, kind="ExternalInput").ap()
    wvT = nc.dram_tensor("wvT", [c, cp], BF, kind="ExternalInput").ap()
    wpT = nc.dram_tensor("wpT", [cp, c], BF, kind="ExternalInput").ap()
    maskT = nc.dram_tensor("maskT", [n, n], BF, kind="ExternalInput").ap()
    out = nc.dram_tensor("out", [n, c], BF, kind="ExternalOutput").ap()
    d2dr = nc.dram_tensor("d2scratch", [2, cp // hd // 2, n], FP, kind="Internal").ap()

    with tile.TileContext(nc) as tc:
        with (
            tc.tile_pool(name="persist", bufs=1) as pers,
        ):
            qT_sb = pers.tile([128, MB, n], BF, tag="qT")
            kT_sb = pers.tile([128, MB, n], BF, tag="kT")
            vaug_sb = pers.tile([128, NB, hpc * hd1], BF, tag="vaug")
            mask_sb = pers.tile([128, NB, n], BF, tag="maskT")
            aoT_sb = pers.tile([128, MB, n], BF, tag="aoT")
            wp_sb = pers.tile([128, MB, c], BF, tag="wp")

            # ---- input DMAs.  x/weights on the sync queue; the (large) mask
            # on the gpsimd queue so both streams progress in parallel.
            xr = xT.rearrange("(cb p) n -> p cb n", p=128)
            mr = maskT.rearrange("(kb p) q -> p kb q", p=128)

            # ---------------- Phase 1: QKV projections ----------------
            with (
                tc.tile_pool(name="xt", bufs=1) as xpool,
                tc.tile_pool(name="w", bufs=1) as wpool,
                tc.tile_pool(name="ps_qkv", bufs=6, space="PSUM") as pq,
            ):
                xT_sb = xpool.tile([128, CB, n], BF, tag="xT")
                w_sb = {}
                for wn, ap_ in (("q", wqT), ("k", wkT)):
                    wt = wpool.tile([128, CB, cp], BF, tag="w" + wn)
                    nc.sync.dma_start(wt, ap_.rearrange("(cb p) m -> p cb m", p=128))
                    w_sb[wn] = wt
                for cb in range(CB):
                    nc.sync.dma_start(xT_sb[:, cb, :], xr[:, cb, :])
                wt = wpool.tile([128, CB, cp], BF, tag="wv")
                nc.sync.dma_start(wt, wvT.rearrange("(cb p) m -> p cb m", p=128))
                w_sb["v"] = wt
                nc.sync.dma_start(wp_sb, wpT.rearrange("(mb p) co -> p mb co", p=128))
                for kb in range(NB):
                    nc.gpsimd.dma_start(mask_sb[:, kb, :], mr[:, kb, :])

                # qT/kT: out [c' block, n]; lhsT = w chunk, rhs = xT chunk
                for wn, dst in (("q", qT_sb), ("k", kT_sb)):
                    for mb in range(MB):
                        for qc in range(QC):
                            pt = pq.tile([128, 512], FP, tag="psqkv")
                            for cb in range(CB):
                                nc.tensor.matmul(
                                    pt,
                                    lhsT=w_sb[wn][:, cb, mb * 128:(mb + 1) * 128],
                                    rhs=xT_sb[:, cb, qc * 512:(qc + 1) * 512],
                                    start=(cb == 0),
                                    stop=(cb == CB - 1),
                                )
                            nc.scalar.copy(
                                dst[:, mb, qc * 512:(qc + 1) * 512], pt
                            )
                # v: natural layout [n block, c']; lhsT = xT chunk, rhs = wvT
                for nb in range(NB):
                    pt = pq.tile([128, cp], FP, tag="psqkv")
                    for cb in range(CB):
                        nc.tensor.matmul(
                            pt,
                            lhsT=xT_sb[:, cb, nb * 128:(nb + 1) * 128],
                            rhs=w_sb["v"][:, cb, :],
                            start=(cb == 0),
                            stop=(cb == CB - 1),
                        )
                    dst3 = vaug_sb[:, nb, :].rearrange("p (h e) -> p h e", e=hd1)
                    nc.scalar.copy(
                        dst3[:, :, 0:hd],
                        pt.rearrange("p (h e) -> p h e", e=hd),
                    )
                    nc.vector.memset(dst3[:, :, hd:hd1], 1.0)

            # ---------------- Phase 2: scores / softmax / attn@V ------------
            with tc.tile_pool(name="d2p", bufs=1) as d2p:
                ones_raw = d2p.tile([1, hd], FP, tag="ones_raw")
                nc.vector.memset(ones_raw, 1.0)
                ones_sb = d2p.tile([1, hd], FR, tag="ones")
                nc.vector.tensor_copy(ones_sb, ones_raw)
                with (
                    tc.tile_pool(name="ps_sc", bufs=2, space="PSUM") as psc,
                    tc.tile_pool(name="ps_ao", bufs=2, space="PSUM") as pao,
                    tc.tile_pool(name="s_exp", bufs=4) as sep,
                    tc.tile_pool(name="s_m", bufs=6) as smp,
                    tc.tile_pool(name="dtp", bufs=4) as dtp,
                ):
                    for qh in range(QH):
                        qo = qh * QW
                        for hb in range(MB):
                            ha, hb_ = 2 * hb, 2 * hb + 1  # pair: rows 0-63/64-127
                            ao = {}
                            for h, po in ((ha, 0), (hb_, 64)):
                                ao[h] = pao.tile([hd1, QW], FP, tag="ao", name="ao")
                            sc_of = {}
                            for kb in range(NB):
                                # scores, pair-interleaved (concurrent row groups)
                                sc_of[ha] = psc.tile([128, QW], FP, tag="sc", name="sca")
                                sc_of[hb_] = psc.tile([128, QW], FP, tag="sc", name="scb")
                                for qs in range(QW // 512):
                                    for h, po in ((ha, 0), (hb_, 64)):
                                        nc.tensor.matmul(
                                            sc_of[h][:, qs * 512:(qs + 1) * 512],
                                            lhsT=kT_sb[po:po + hd, hb, kb * 128:(kb + 1) * 128],
                                            rhs=qT_sb[po:po + hd, hb, qo + qs * 512:qo + (qs + 1) * 512],
                                            start=True,
                                            stop=True,
                                        )
                                for h, po in ((ha, 0), (hb_, 64)):
                                    se = sep.tile([128, QW], BF, tag="se")
                                    nc.scalar.activation(se, sc_of[h], EXP, scale=scale)
                                    sm = smp.tile([128, QW], BF, tag="sm")
                                    nc.vector.tensor_mul(
                                        sm, se, mask_sb[:, kb, qo:qo + QW]
                                    )
                                    for qs in range(QW // 512):
                                        nc.tensor.matmul(
                                            ao[h][:, qs * 512:(qs + 1) * 512],
                                            lhsT=vaug_sb[:, kb, h * hd1:(h + 1) * hd1],
                                            rhs=sm[:, qs * 512:(qs + 1) * 512],
                                            start=(kb == 0),
                                            stop=(kb == NB - 1),
                                        )
                            # ---- chunk epilogue: evacuate ao; stage the
                            # denominator row (PSUM p64 -> dtmp p0 -> DRAM).
                            for par, (h, po) in enumerate(((ha, 0), (hb_, 64))):
                                dtmp = dtp.tile([1, QW], FP, tag="dtmp", name="dt")
                                nc.vector.tensor_copy(dtmp, ao[h][hd:hd1, :])
                                nc.vector.tensor_copy(
                                    aoT_sb[po:po + hd, hb, qo:qo + QW], ao[h][0:hd, :]
                                )
                                nc.scalar.dma_start(
                                    d2dr[par:par + 1, hb, qo:qo + QW], dtmp
                                )

                # ---- tail: normalization broadcast + output projection ----
                with tc.tile_pool(name="dnp", bufs=1) as dnp:
                    dinv2 = dnp.tile([2, MB, n], FP, tag="dinv2")
                    with tc.tile_pool(name="d2tp", bufs=1) as d2tp:
                        d2t = d2tp.tile([2, MB, n], FP, tag="d2t")
                        nc.scalar.dma_start(d2t, d2dr)
                        nc.vector.reciprocal_approx_fast(dinv2, d2t)
                    with (
                        tc.tile_pool(name="d0p", bufs=2) as d0p,
                        tc.tile_pool(name="ps_bc", bufs=2, space="PSUM") as pbc,
                    ):
                        for hb in range(MB):
                            # stage both parity rows as fp32r at partition 0
                            # (DMA rounds via the FR-typed destination)
                            d0a = d0p.tile([1, n], FR, tag="d0a", name="d0a")
                            d0b = d0p.tile([1, n], FR, tag="d0b", name="d0b")
                            nc.scalar.dma_start(d0a, dinv2[0:1, hb, :].bitcast(FR))
                            nc.scalar.dma_start(d0b, dinv2[1:2, hb, :].bitcast(FR))
                            bca = pbc.tile([hd, n], FP, tag="bc", name="bca")
                            bcb = pbc.tile([hd, n], FP, tag="bc", name="bcb")
                            for qc in range(QC):
                                cs_ = slice(qc * 512, (qc + 1) * 512)
                                nc.tensor.matmul(
                                    bca[:, cs_], lhsT=ones_sb, rhs=d0a[:, cs_],
                                    start=True, stop=True,
                                )
                                nc.tensor.matmul(
                                    bcb[:, cs_], lhsT=ones_sb, rhs=d0b[:, cs_],
                                    start=True, stop=True,
                                )
                            nc.vector.tensor_mul(
                                aoT_sb[0:hd, hb, :], aoT_sb[0:hd, hb, :], bca
                            )
                            nc.vector.tensor_mul(
                                aoT_sb[hd:128, hb, :], aoT_sb[hd:128, hb, :], bcb
                            )

                    with (
                        tc.tile_pool(name="ps_o", bufs=4, space="PSUM") as pso,
                        tc.tile_pool(name="osb", bufs=3) as osp,
                    ):
                        for nb in range(NB):
                            ot = osp.tile([128, c], BF, tag="ot")
                            for co in range(c // 512):
                                pt = pso.tile([128, 512], FP, tag="pso")
                                for mb in range(MB):
                                    nc.tensor.matmul(
                                        pt,
                                        lhsT=aoT_sb[:, mb, nb * 128:(nb + 1) * 128],
                                        rhs=wp_sb[:, mb, co * 512:(co + 1) * 512],
                                        start=(mb == 0),
                                        stop=(mb == MB - 1),
                                    )
                                # split evacuations between DVE and ScalarE
                                if co == 0:
                                    nc.vector.tensor_copy(
                                        ot[:, co * 512:(co + 1) * 512], pt
                                    )
                                else:
                                    nc.scalar.copy(ot[:, co * 512:(co + 1) * 512], pt)
                            nc.sync.dma_start(
                                out.rearrange("(nb p) co -> p nb co", p=128)[:, nb, :],
                                ot,
                            )
    nc.compile()
    return nc


def make_in_maps(x, mask, wq, wk, wv, wp):
    """Host-side sharding: per-core input dict (bf16)."""
    bf16 = ml_dtypes.bfloat16
    xTb = [np.ascontiguousarray(x[b].T).astype(bf16) for b in range(B)]
    mTb = [np.ascontiguousarray(mask[b].T).astype(bf16) for b in range(B)]
    in_maps = []
    for core in range(NCORES):
        b = core // 2
        g = core % 2
        cs = slice(g * 512, (g + 1) * 512)
        in_maps.append({
            "xT": xTb[b],
            "wqT": np.ascontiguousarray(wq[cs, :].T).astype(bf16),
            "wkT": np.ascontiguousarray(wk[cs, :].T).astype(bf16),
            "wvT": np.ascontiguousarray(wv[cs, :].T).astype(bf16),
            "wpT": np.ascontiguousarray(wp[:, cs].T).astype(bf16),
            "maskT": mTb[b],
        })
    return in_maps


_NC_CACHE = {}


def _get_nc():
    if "nc" not in _NC_CACHE:
        _NC_CACHE["nc"] = build_attention()
    return _NC_CACHE["nc"]


def kernel(x, mask, wq, wk, wv, wp, bp, _trace=False, _trace_kwargs=None):
    x = np.asarray(x, dtype=np.float32)
    mask = np.asarray(mask)
    wq = np.asarray(wq, dtype=np.float32)
    wk = np.asarray(wk, dtype=np.float32)
    wv = np.asarray(wv, dtype=np.float32)
    wp = np.asarray(wp, dtype=np.float32)
    bp = np.asarray(bp, dtype=np.float32)

    nc = _get_nc()
    in_maps = make_in_maps(x, mask, wq, wk, wv, wp)
    kw = {}
    if _trace:
        kw = {"trace": True, **(_trace_kwargs or {})}
    res = run_bass_kernel_spmd(nc, in_maps, list(range(NCORES)), **kw)
    outs = [np.asarray(r["out"], dtype=np.float32) for r in res.results]
    full = np.empty((B, N, DIM), dtype=np.float32)
    for b in range(B):
        full[b] = outs[2 * b] + outs[2 * b + 1] + bp[None, :]
    if _trace:
        return full, res
    return full


if __name__ == "__main__":
    nc = build_attention()
    print("built ok")


# revision 7
# speedup vs baseline: 1.3898x; 1.3898x over previous
"""Multi-head attention (B=4, N=2048, C=1024, H=16) on 8 trn2 NeuronCores.

Sharding: data-parallel over batch (4) x tensor-parallel over heads (2).
Core c handles batch c//2, heads [8*(c%2), 8*(c%2)+8). Each core computes a
partial output projection; the host sums core pairs and adds the bias.

v2 vs baseline:
- All inputs bf16 (x, wq/wk/wv/wp, mask); output bf16, summed in fp32 on
  host. Halves DMA traffic; matmul rate unchanged.
- Input DMAs staggered: weights first, xT split across the sync and gpsimd
  queues, the (8MB) mask follows on the gpsimd queue; full mask resident in
  SBUF so phase 2 never waits on mask DMA.
- Phase-1 PSUM evacuations on ScalarE (idle during phase 1); exp table
  preloaded during phase 1 via a dummy activation.
- Phase 2 keeps the baseline's proven software pipeline (LOOK=2, 3 score
  buffers, head-outer units) at QW=1024.
- Denominator rows collected into d_sb[8, n] per head via dtmp+DMA (as
  baseline), but the tail normalization uses K=2 pair-select matmuls (host
  supplies the [2,128] selector) -> one [128, n] broadcast + one TT per
  head block instead of per-head [64, n] passes.
"""

import os
import sys

for p in ("/opt/trn_rl_repo", "/root/.axon_site/_ro/trn_rl_repo"):
    if os.path.isdir(p) and p not in sys.path:
        sys.path.insert(0, p)

import ml_dtypes
import numpy as np

import concourse.bacc as bacc
import concourse.tile as tile
from concourse import mybir
from concourse.bass_utils import run_bass_kernel_spmd

FP = mybir.dt.float32
FR = mybir.dt.float32r
BF = mybir.dt.bfloat16
EXP = mybir.ActivationFunctionType.Exp

DIM = 1024
NUM_HEADS = 16
HEAD_DIM = 64
SCALE = HEAD_DIM ** -0.5
B, N = 4, 2048
NCORES = 8


def build_attention(n=N, c=DIM, cp=DIM // 2, hd=HEAD_DIM, scale=SCALE):
    """Emit the per-core program. All cores run the same code (SPMD)."""
    hpc = cp // hd          # heads on this core (8)
    CB = c // 128           # contraction blocks for QKV (8)
    MB = cp // 128          # c' blocks (q/k transposed layout) (4)
    NB = n // 128           # token/key blocks (16)
    QC = n // 512           # 512-col chunks (4)
    QW = 1024               # phase-2 q chunk width
    QH = n // QW            # phase-2 outer q chunks (2)
    hd1 = hd + 1            # v augmented with ones column -> denominator

    nc = bacc.Bacc("TRN2", target_bir_lowering=False, debug=False)

    xT = nc.dram_tensor("xT", [c, n], BF, kind="ExternalInput").ap()
    wqT = nc.dram_tensor("wqT", [c, cp], BF, kind="ExternalInput").ap()
    wkT = nc.dram_tensor("wkT", [c, cp], BF, kind="ExternalInput").ap()
    wvT = nc.dram_tensor("wvT", [c, cp], BF, kind="ExternalInput").ap()
    wpT = nc.dram_tensor("wpT", [cp, c], BF, kind="ExternalInput").ap()
    maskT = nc.dram_tensor("maskT", [n, n], BF, kind="ExternalInput").ap()
    psel = nc.dram_tensor("psel", [2, 128], FP, kind="ExternalInput").ap()
    out = nc.dram_tensor("out", [n, c], BF, kind="ExternalOutput").ap()

    with tile.TileContext(nc) as tc:
        with (
            tc.tile_pool(name="persist", bufs=1) as pers,
            tc.tile_pool(name="d_pool", bufs=1) as dpool,
        ):
            qT_sb = pers.tile([128, MB, n], BF, tag="qT")
            kT_sb = pers.tile([128, MB, n], BF, tag="kT")
            vaug_sb = pers.tile([128, NB, hpc * hd1], BF, tag="vaug")
            mask_sb = pers.tile([128, NB, n], BF, tag="maskT")
            aoT_sb = pers.tile([128, MB, n], BF, tag="aoT")
            wp_sb = pers.tile([128, MB, c], BF, tag="wp")
            psel_sb = pers.tile([2, 128], FR, tag="psel")
            d_sb = dpool.tile([hpc, n], FP, tag="dsum")

            xr = xT.rearrange("(cb p) n -> p cb n", p=128)
            mr = maskT.rearrange("(kb p) q -> p kb q", p=128)

            # ---------------- Phase 1: QKV projections ----------------
            with (
                tc.tile_pool(name="xt", bufs=1) as xpool,
                tc.tile_pool(name="w", bufs=1) as wpool,
                tc.tile_pool(name="warm", bufs=1) as warmp,
                tc.tile_pool(name="ps_qkv", bufs=6, space="PSUM") as pq,
            ):
                # --- staggered input DMAs: weights + x first (split across
                # two queues), then the mask streams behind on gpsimd.
                xT_sb = xpool.tile([128, CB, n], BF, tag="xT")
                w_sb = {}
                for wn, ap_ in (("q", wqT), ("k", wkT)):
                    wt = wpool.tile([128, CB, cp], BF, tag="w" + wn)
                    nc.sync.dma_start(wt, ap_.rearrange("(cb p) m -> p cb m", p=128))
                    w_sb[wn] = wt
                for cb in range(CB):
                    q_ = nc.sync if cb % 2 == 0 else nc.gpsimd
                    q_.dma_start(xT_sb[:, cb, :], xr[:, cb, :])
                wt = wpool.tile([128, CB, cp], BF, tag="wv")
                nc.sync.dma_start(wt, wvT.rearrange("(cb p) m -> p cb m", p=128))
                w_sb["v"] = wt
                nc.sync.dma_start(wp_sb, wpT.rearrange("(mb p) co -> p mb co", p=128))
                nc.sync.dma_start(psel_sb, psel.bitcast(FR))
                for kb in range(NB):
                    nc.gpsimd.dma_start(mask_sb[:, kb, :], mr[:, kb, :])

                # preload the exp table set off the critical path
                wtile = warmp.tile([1, 16], FP, tag="warm")
                nc.vector.memset(wtile, 0.0)
                wout = warmp.tile([1, 16], BF, tag="warmo")
                nc.scalar.activation(wout, wtile, EXP, scale=1.0)

                # qT/kT: out [c' block, n]; lhsT = w chunk, rhs = xT chunk
                for wn, dst in (("q", qT_sb), ("k", kT_sb)):
                    for mb in range(MB):
                        for qc in range(QC):
                            pt = pq.tile([128, 512], FP, tag="psqkv")
                            for cb in range(CB):
                                nc.tensor.matmul(
                                    pt,
                                    lhsT=w_sb[wn][:, cb, mb * 128:(mb + 1) * 128],
                                    rhs=xT_sb[:, cb, qc * 512:(qc + 1) * 512],
                                    start=(cb == 0),
                                    stop=(cb == CB - 1),
                                )
                            nc.scalar.copy(
                                dst[:, mb, qc * 512:(qc + 1) * 512], pt
                            )
                # v: natural layout [n block, c']; lhsT = xT chunk, rhs = wvT
                for nb in range(NB):
                    pt = pq.tile([128, cp], FP, tag="psqkv")
                    for cb in range(CB):
                        nc.tensor.matmul(
                            pt,
                            lhsT=xT_sb[:, cb, nb * 128:(nb + 1) * 128],
                            rhs=w_sb["v"][:, cb, :],
                            start=(cb == 0),
                            stop=(cb == CB - 1),
                        )
                    dst3 = vaug_sb[:, nb, :].rearrange("p (h e) -> p h e", e=hd1)
                    nc.scalar.copy(
                        dst3[:, :, 0:hd],
                        pt.rearrange("p (h e) -> p h e", e=hd),
                    )
                    nc.vector.memset(dst3[:, :, hd:hd1], 1.0)

            # ---------------- Phase 2: scores / softmax / attn@V ------------
            # Baseline-proven software pipeline: units (h, kb), scores
            # emitted LOOK units ahead, 3 rotating score tiles, 1 ao tile.
            with (
                tc.tile_pool(name="ps_sc", bufs=3, space="PSUM") as psc,
                tc.tile_pool(name="ps_ao", bufs=1, space="PSUM") as pao,
                tc.tile_pool(name="s_exp", bufs=6) as sep,
                tc.tile_pool(name="s_m", bufs=6) as smp,
                tc.tile_pool(name="dtp", bufs=2) as dtp,
            ):
                for qh in range(QH):
                    qo = qh * QW
                    units = [(h, kb) for h in range(hpc) for kb in range(NB)]
                    LOOK = 2
                    sc_map = {}
                    ao_map = {}
                    for idx in range(len(units) + LOOK):
                        if idx < len(units):
                            h, kb = units[idx]
                            po = (h % 2) * hd
                            hb = h // 2
                            sc = psc.tile([128, QW], FP, tag="sc", name="sc")
                            sc_map[idx] = sc
                            for qs in range(QW // 512):
                                nc.tensor.matmul(
                                    sc[:, qs * 512:(qs + 1) * 512],
                                    lhsT=kT_sb[po:po + hd, hb, kb * 128:(kb + 1) * 128],
                                    rhs=qT_sb[po:po + hd, hb, qo + qs * 512:qo + (qs + 1) * 512],
                                    start=True,
                                    stop=True,
                                )
                        j = idx - LOOK
                        if j < 0:
                            continue
                        h, kb = units[j]
                        po = (h % 2) * hd
                        hb = h // 2
                        sc = sc_map.pop(j)
                        se = sep.tile([128, QW], BF, tag="se")
                        nc.scalar.activation(se, sc, EXP, scale=scale)
                        sm = smp.tile([128, QW], BF, tag="sm")
                        nc.vector.tensor_mul(sm, se, mask_sb[:, kb, qo:qo + QW])
                        if kb == 0:
                            ao_map[h] = pao.tile([hd1, QW], FP, tag="ao", name="ao")
                        ao = ao_map[h]
                        for qs in range(QW // 512):
                            nc.tensor.matmul(
                                ao[:, qs * 512:(qs + 1) * 512],
                                lhsT=vaug_sb[:, kb, h * hd1:(h + 1) * hd1],
                                rhs=sm[:, qs * 512:(qs + 1) * 512],
                                start=(kb == 0),
                                stop=(kb == NB - 1),
                            )
                        if kb == NB - 1:
                            # evacuate + stage denominator row
                            nc.vector.tensor_copy(
                                aoT_sb[po:po + hd, hb, qo:qo + QW], ao[0:hd, :]
                            )
                            dtmp = dtp.tile([1, QW], FP, tag="dtmp", name="dt")
                            nc.vector.tensor_copy(dtmp, ao[hd:hd1, :])
                            nc.scalar.dma_start(d_sb[h:h + 1, qo:qo + QW], dtmp)
                            del ao_map[h]

            # ---- tail: normalization broadcast + output projection ----
            with tc.tile_pool(name="dnp", bufs=1) as dnp:
                dinv = dnp.tile([hpc, n], FP, tag="dinv")
                nc.vector.reciprocal_approx_fast(dinv, d_sb)
                with (
                    tc.tile_pool(name="d0p", bufs=2) as d0p,
                    tc.tile_pool(name="ps_bc", bufs=2, space="PSUM") as pbc,
                ):
                    for hb in range(MB):
                        # stage the head pair's dinv rows to partitions 0/1
                        d0 = d0p.tile([2, n], FR, tag="d0", name="d0")
                        nc.scalar.dma_start(
                            d0[0:1, :], dinv[2 * hb:2 * hb + 1, :].bitcast(FR)
                        )
                        nc.scalar.dma_start(
                            d0[1:2, :], dinv[2 * hb + 1:2 * hb + 2, :].bitcast(FR)
                        )
                        bc = pbc.tile([128, n], FP, tag="bc", name="bc")
                        for qc in range(QC):
                            cs_ = slice(qc * 512, (qc + 1) * 512)
                            nc.tensor.matmul(
                                bc[:, cs_], lhsT=psel_sb, rhs=d0[:, cs_],
                                start=True, stop=True,
                            )
                        nc.vector.tensor_mul(
                            aoT_sb[:, hb, :], aoT_sb[:, hb, :], bc
                        )

                with (
                    tc.tile_pool(name="ps_o", bufs=4, space="PSUM") as pso,
                    tc.tile_pool(name="osb", bufs=3) as osp,
                ):
                    for nb in range(NB):
                        ot = osp.tile([128, c], BF, tag="ot")
                        for co in range(c // 512):
                            pt = pso.tile([128, 512], FP, tag="pso")
                            for mb in range(MB):
                                nc.tensor.matmul(
                                    pt,
                                    lhsT=aoT_sb[:, mb, nb * 128:(nb + 1) * 128],
                                    rhs=wp_sb[:, mb, co * 512:(co + 1) * 512],
                                    start=(mb == 0),
                                    stop=(mb == MB - 1),
                                )
                            if co == 0:
                                nc.vector.tensor_copy(
                                    ot[:, co * 512:(co + 1) * 512], pt
                                )
                            else:
                                nc.scalar.copy(ot[:, co * 512:(co + 1) * 512], pt)
                        nc.sync.dma_start(
                            out.rearrange("(nb p) co -> p nb co", p=128)[:, nb, :],
                            ot,
                        )
    nc.compile()
    return nc


def make_in_maps(x, mask, wq, wk, wv, wp):
    """Host-side sharding: per-core input dict (bf16)."""
    bf16 = ml_dtypes.bfloat16
    pselv = np.zeros((2, 128), dtype=np.float32)
    pselv[0, 0:64] = 1.0
    pselv[1, 64:128] = 1.0
    xTb = [np.ascontiguousarray(x[b].T).astype(bf16) for b in range(B)]
    mTb = [np.ascontiguousarray(mask[b].T).astype(bf16) for b in range(B)]
    in_maps = []
    for core in range(NCORES):
        b = core // 2
        g = core % 2
        cs = slice(g * 512, (g + 1) * 512)
        in_maps.append({
            "xT": xTb[b],
            "wqT": np.ascontiguousarray(wq[cs, :].T).astype(bf16),
            "wkT": np.ascontiguousarray(wk[cs, :].T).astype(bf16),
            "wvT": np.ascontiguousarray(wv[cs, :].T).astype(bf16),
            "wpT": np.ascontiguousarray(wp[:, cs].T).astype(bf16),
            "maskT": mTb[b],
            "psel": pselv,
        })
    return in_maps


_NC_CACHE = {}


def _get_nc():
    if "nc" not in _NC_CACHE:
        _NC_CACHE["nc"] = build_attention()
    return _NC_CACHE["nc"]


def kernel(x, mask, wq, wk, wv, wp, bp, _trace=False, _trace_kwargs=None):
    x = np.asarray(x, dtype=np.float32)
    mask = np.asarray(mask)
    wq = np.asarray(wq, dtype=np.float32)
    wk = np.asarray(wk, dtype=np.float32)
    wv = np.asarray(wv, dtype=np.float32)
    wp = np.asarray(wp, dtype=np.float32)
    bp = np.asarray(bp, dtype=np.float32)

    nc = _get_nc()
    in_maps = make_in_maps(x, mask, wq, wk, wv, wp)
    kw = {}
    if _trace:
        kw = {"trace": True, **(_trace_kwargs or {})}
    res = run_bass_kernel_spmd(nc, in_maps, list(range(NCORES)), **kw)
    outs = [np.asarray(r["out"], dtype=np.float32) for r in res.results]
    full = np.empty((B, N, DIM), dtype=np.float32)
    for b in range(B):
        full[b] = outs[2 * b] + outs[2 * b + 1] + bp[None, :]
    if _trace:
        return full, res
    return full


if __name__ == "__main__":
    nc = build_attention()
    print("built ok")


# revision 8
# speedup vs baseline: 1.4040x; 1.0102x over previous
"""Multi-head attention (B=4, N=2048, C=1024, H=16) on 8 trn2 NeuronCores.

Sharding: data-parallel over batch (4) x tensor-parallel over heads (2).
Core c handles batch c//2, heads [8*(c%2), 8*(c%2)+8). Each core computes a
partial output projection; the host sums core pairs and adds the bias.

v3: phase 2 is ScalarE-bound (256 exp ACTIVATEs of [128,1024]), so the
remaining QKV projection work is hidden under it:
- Unit order is head-block-OUTER ((hb, qh, h, kb)): q/k projections for
  blocks 1-3 (and v blocks 8-15) INJECT into phase 2's TensorE slack,
  accumulating in half-width slices of the rotating score PSUM tiles.
  Injected q/k groups re-stream their xT slice from DRAM (prefetched one
  injection ahead on the gpsimd queue) so xT need not stay in SBUF.
- Serial prologue: only q/k block 0 + v blocks 0..7.
- Input DMAs: critical loads first, the 8MB mask queued BEHIND them.
- Phase-1/prologue evacuations on ScalarE; injected evacuations on VectorE.
- exp table preloaded early; LOOK=2 score pipeline (3 rotating sc tiles).
- Denominator rows staged via dtmp->DMA on the gpsimd queue; epilogue
  emission delayed past the next unit's mask-mul.
- Tail: one reciprocal, K=2 pair-select broadcast matmuls, 4 wide in-place
  normalizations, projection (evacs split V/S), out DMAs on both queues.
"""

import os
import sys

for p in ("/opt/trn_rl_repo", "/root/.axon_site/_ro/trn_rl_repo"):
    if os.path.isdir(p) and p not in sys.path:
        sys.path.insert(0, p)

import ml_dtypes
import numpy as np

import concourse.bacc as bacc
import concourse.tile as tile
from concourse import mybir
from concourse.bass_utils import run_bass_kernel_spmd

FP = mybir.dt.float32
FR = mybir.dt.float32r
BF = mybir.dt.bfloat16
EXP = mybir.ActivationFunctionType.Exp

DIM = 1024
NUM_HEADS = 16
HEAD_DIM = 64
SCALE = HEAD_DIM ** -0.5
B, N = 4, 2048
NCORES = 8


def build_attention(n=N, c=DIM, cp=DIM // 2, hd=HEAD_DIM, scale=SCALE):
    """Emit the per-core program. All cores run the same code (SPMD)."""
    hpc = cp // hd          # heads on this core (8)
    CB = c // 128           # contraction blocks for QKV (8)
    MB = cp // 128          # c' blocks (4)
    NB = n // 128           # token/key blocks (16)
    QC = n // 512           # 512-col chunks (4)
    QW = 1024               # phase-2 q chunk width
    QH = n // QW            # phase-2 outer q chunks (2)
    hd1 = hd + 1            # v augmented with ones column
    PV = 8                  # v blocks computed in the serial prologue

    nc = bacc.Bacc("TRN2", target_bir_lowering=False, debug=False)

    xT = nc.dram_tensor("xT", [c, n], BF, kind="ExternalInput").ap()
    wqT = nc.dram_tensor("wqT", [c, cp], BF, kind="ExternalInput").ap()
    wkT = nc.dram_tensor("wkT", [c, cp], BF, kind="ExternalInput").ap()
    wvT = nc.dram_tensor("wvT", [c, cp], BF, kind="ExternalInput").ap()
    wpT = nc.dram_tensor("wpT", [cp, c], BF, kind="ExternalInput").ap()
    maskT = nc.dram_tensor("maskT", [n, n], BF, kind="ExternalInput").ap()
    psel = nc.dram_tensor("psel", [2, 128], FP, kind="ExternalInput").ap()
    out = nc.dram_tensor("out", [n, c], BF, kind="ExternalOutput").ap()

    xr = xT.rearrange("(cb p) n -> p cb n", p=128)
    mr = maskT.rearrange("(kb p) q -> p kb q", p=128)

    with tile.TileContext(nc) as tc:
        with (
            tc.tile_pool(name="persist", bufs=1) as pers,
            tc.tile_pool(name="d_pool", bufs=1) as dpool,
        ):
            qT_sb = pers.tile([128, MB, n], BF, tag="qT")
            kT_sb = pers.tile([128, MB, n], BF, tag="kT")
            vaug_sb = pers.tile([128, NB, hpc * hd1], BF, tag="vaug")
            mask_sb = pers.tile([128, NB, n], BF, tag="maskT")
            aoT_sb = pers.tile([128, MB, n], BF, tag="aoT")
            wp_sb = pers.tile([128, MB, c], BF, tag="wp")
            psel_sb = pers.tile([2, 128], FR, tag="psel")
            wq_sb = pers.tile([128, CB, cp], BF, tag="wqs")
            wk_sb = pers.tile([128, CB, cp], BF, tag="wks")
            wv_sb = pers.tile([128, CB, cp], BF, tag="wvs")
            d_sb = dpool.tile([hpc, n], FP, tag="dsum")

            def qk_mms(w_t, x_t, mb, dst_cols, pt, evac_engine):
                """q/k projection group; x_t columns already sliced."""
                dst, wn_t = dst_cols
                for cb in range(CB):
                    nc.tensor.matmul(
                        pt,
                        lhsT=w_t[:, cb, mb * 128:(mb + 1) * 128],
                        rhs=x_t[:, cb, :],
                        start=(cb == 0),
                        stop=(cb == CB - 1),
                    )
                evac_engine(dst, pt)

            def v_mms(x_t, wv_t, pt, nb, evac_engine):
                for cb in range(CB):
                    nc.tensor.matmul(
                        pt,
                        lhsT=x_t[:, cb, :],
                        rhs=wv_t[:, cb, :],
                        start=(cb == 0),
                        stop=(cb == CB - 1),
                    )
                dst3 = vaug_sb[:, nb, :].rearrange("p (h e) -> p h e", e=hd1)
                evac_engine(
                    dst3[:, :, 0:hd], pt.rearrange("p (h e) -> p h e", e=hd)
                )
                nc.vector.memset(dst3[:, :, hd:hd1], 1.0)

            # ---------------- Prologue ----------------
            with (
                tc.tile_pool(name="xt", bufs=1) as xpool,
                tc.tile_pool(name="warm", bufs=1) as warmp,
                tc.tile_pool(name="ps_qkv", bufs=4, space="PSUM") as pq,
            ):
                xT_sb = xpool.tile([128, CB, n], BF, tag="xT")

                # critical loads first; mask queued BEHIND them on sync.
                nc.sync.dma_start(wq_sb, wqT.rearrange("(cb p) m -> p cb m", p=128))
                for cb in range(0, CB, 2):
                    nc.sync.dma_start(xT_sb[:, cb, :], xr[:, cb, :])
                for cb in range(1, CB, 2):
                    nc.gpsimd.dma_start(xT_sb[:, cb, :], xr[:, cb, :])
                nc.sync.dma_start(wk_sb, wkT.rearrange("(cb p) m -> p cb m", p=128))
                nc.gpsimd.dma_start(
                    wv_sb, wvT.rearrange("(cb p) m -> p cb m", p=128)
                )
                nc.gpsimd.dma_start(psel_sb, psel.bitcast(FR))
                for kb in range(NB):
                    q_ = nc.sync if kb % 2 == 0 else nc.gpsimd
                    q_.dma_start(mask_sb[:, kb, :], mr[:, kb, :])
                nc.sync.dma_start(wp_sb, wpT.rearrange("(mb p) co -> p mb co", p=128))

                # preload the exp table set off the critical path
                wtile = warmp.tile([1, 16], FP, tag="warm")
                nc.vector.memset(wtile, 0.0)
                wout = warmp.tile([1, 16], BF, tag="warmo")
                nc.scalar.activation(wout, wtile, EXP, scale=1.0)

                for wn, w_t, dst in (("q", wq_sb, qT_sb), ("k", wk_sb, kT_sb)):
                    for qc in range(QC):
                        pt = pq.tile([128, 512], FP, tag="psqkv")
                        qk_mms(
                            w_t, xT_sb[:, :, qc * 512:(qc + 1) * 512], 0,
                            (dst[:, 0, qc * 512:(qc + 1) * 512], wn), pt,
                            nc.scalar.copy,
                        )
                for nb in range(PV):
                    pt = pq.tile([128, cp], FP, tag="psqkv")
                    v_mms(
                        xT_sb[:, :, nb * 128:(nb + 1) * 128], wv_sb, pt, nb,
                        nc.scalar.copy,
                    )

            # ---------------- Phase 2 with injected QKV ----------------
            # injections: v blocks PV..15 first (needed at unit kb), then
            # q/k pairs (mb 1..3) whose xT slice is re-streamed from DRAM.
            inject = [("v", nb) for nb in range(PV, NB)]
            for mb in range(1, MB):
                for qc in range(QC):
                    inject.append(("qk", mb, qc))  # q+k share the x slice

            with (
                tc.tile_pool(name="xq", bufs=2) as xqp,
                tc.tile_pool(name="ps_sc", bufs=3, space="PSUM") as psc,
                tc.tile_pool(name="ps_ao", bufs=1, space="PSUM") as pao,
                tc.tile_pool(name="s_exp", bufs=3) as sep,
                tc.tile_pool(name="s_m", bufs=4) as smp,
                tc.tile_pool(name="dtp", bufs=2) as dtp,
            ):
                # prefetch tiles for injections (rotating)
                inj_tiles = {}

                def prefetch_inject(i):
                    if i >= len(inject):
                        return
                    g = inject[i]
                    if g[0] == "v":
                        nb = g[1]
                        t = xqp.tile([128, CB, 512], BF, tag="xq", name="xq")
                        t = t[:, :, 0:128]
                        nc.gpsimd.dma_start(
                            t, xr[:, :, nb * 128:(nb + 1) * 128]
                        )
                    else:
                        qc = g[2]
                        t = xqp.tile([128, CB, 512], BF, tag="xq", name="xq")
                        nc.gpsimd.dma_start(
                            t, xr[:, :, qc * 512:(qc + 1) * 512]
                        )
                    inj_tiles[i] = t

                def emit_inject(i):
                    g = inject[i]
                    t = inj_tiles.pop(i)
                    if g[0] == "v":
                        gt = psc.tile([128, QW], FP, tag="sc", name="inj")
                        v_mms(t, wv_sb, gt[:, 0:cp], g[1], nc.vector.tensor_copy)
                    else:
                        mb, qc = g[1], g[2]
                        for wn, w_t, dst in (
                            ("q", wq_sb, qT_sb), ("k", wk_sb, kT_sb)
                        ):
                            gt = psc.tile([128, QW], FP, tag="sc", name="inj")
                            qk_mms(
                                w_t, t, mb,
                                (dst[:, mb, qc * 512:(qc + 1) * 512], wn),
                                gt[:, 0:512], nc.vector.tensor_copy,
                            )

                # injection points: v block nb at unit nb-PV (they are
                # needed at unit kb=nb); q/k pair mb at unit 12+16*(mb-1).
                inj_at = {}
                nqk = 0
                for i, g in enumerate(inject):
                    if g[0] == "v":
                        inj_at.setdefault(g[1] - PV, []).append(i)
                    else:
                        inj_at.setdefault(12 + 3 * nqk, []).append(i)
                        nqk += 1

                units = [
                    (hb, qh, 2 * hb + par, kb)
                    for hb in range(MB)
                    for qh in range(QH)
                    for par in range(2)
                    for kb in range(NB)
                ]
                LOOK = 2
                sc_map = {}
                ao_cur = [None]
                pending_epi = [None]
                prefetch_inject(0)
                prefetch_inject(1)
                next_pf = 2

                for idx in range(len(units) + LOOK):
                    if idx < len(units):
                        hb, qh, h, kb = units[idx]
                        qo = qh * QW
                        po = (h % 2) * hd
                        sc = psc.tile([128, QW], FP, tag="sc", name="sc")
                        sc_map[idx] = sc
                        for qs in range(QW // 512):
                            nc.tensor.matmul(
                                sc[:, qs * 512:(qs + 1) * 512],
                                lhsT=kT_sb[po:po + hd, hb, kb * 128:(kb + 1) * 128],
                                rhs=qT_sb[po:po + hd, hb, qo + qs * 512:qo + (qs + 1) * 512],
                                start=True,
                                stop=True,
                            )
                    j = idx - LOOK
                    if j < 0:
                        continue
                    hb, qh, h, kb = units[j]
                    qo = qh * QW
                    po = (h % 2) * hd
                    sc = sc_map.pop(j)
                    se = sep.tile([128, QW], BF, tag="se")
                    nc.scalar.activation(se, sc, EXP, scale=scale)
                    sm = smp.tile([128, QW], BF, tag="sm")
                    nc.vector.tensor_mul(sm, se, mask_sb[:, kb, qo:qo + QW])
                    if pending_epi[0] is not None:
                        pending_epi[0]()
                        pending_epi[0] = None
                    if kb == 0:
                        ao_cur[0] = pao.tile([hd1, QW], FP, tag="ao", name="ao")
                    ao = ao_cur[0]
                    for qs in range(QW // 512):
                        nc.tensor.matmul(
                            ao[:, qs * 512:(qs + 1) * 512],
                            lhsT=vaug_sb[:, kb, h * hd1:(h + 1) * hd1],
                            rhs=sm[:, qs * 512:(qs + 1) * 512],
                            start=(kb == 0),
                            stop=(kb == NB - 1),
                        )
                    for i in inj_at.get(j, ()):
                        emit_inject(i)
                        prefetch_inject(next_pf)
                        next_pf += 1
                    if kb == NB - 1:
                        def make_epi(h=h, po=po, hb=hb, qo=qo, ao=ao):
                            def epi():
                                nc.vector.tensor_copy(
                                    aoT_sb[po:po + hd, hb, qo:qo + QW],
                                    ao[0:hd, :],
                                )
                                dtmp = dtp.tile(
                                    [1, QW], FP, tag="dtmp", name="dt"
                                )
                                nc.vector.tensor_copy(dtmp, ao[hd:hd1, :])
                                nc.gpsimd.dma_start(
                                    d_sb[h:h + 1, qo:qo + QW], dtmp
                                )
                            return epi
                        pending_epi[0] = make_epi()
                if pending_epi[0] is not None:
                    pending_epi[0]()
                    pending_epi[0] = None

            # ---- tail: normalization broadcast + output projection ----
            with tc.tile_pool(name="dnp", bufs=1) as dnp:
                dinv = dnp.tile([hpc, n], FP, tag="dinv")
                nc.vector.reciprocal_approx_fast(dinv, d_sb)
                with (
                    tc.tile_pool(name="d0p", bufs=2) as d0p,
                    tc.tile_pool(name="ps_bc", bufs=2, space="PSUM") as pbc,
                ):
                    for hb in range(MB):
                        d0 = d0p.tile([2, n], FR, tag="d0", name="d0")
                        nc.scalar.dma_start(
                            d0[0:1, :], dinv[2 * hb:2 * hb + 1, :].bitcast(FR)
                        )
                        nc.scalar.dma_start(
                            d0[1:2, :], dinv[2 * hb + 1:2 * hb + 2, :].bitcast(FR)
                        )
                        bc = pbc.tile([128, n], FP, tag="bc", name="bc")
                        for qc in range(QC):
                            cs_ = slice(qc * 512, (qc + 1) * 512)
                            nc.tensor.matmul(
                                bc[:, cs_], lhsT=psel_sb, rhs=d0[:, cs_],
                                start=True, stop=True,
                            )
                        nc.vector.tensor_mul(
                            aoT_sb[:, hb, :], aoT_sb[:, hb, :], bc
                        )

                with (
                    tc.tile_pool(name="ps_o", bufs=4, space="PSUM") as pso,
                    tc.tile_pool(name="osb", bufs=3) as osp,
                ):
                    for nb in range(NB):
                        ot = osp.tile([128, c], BF, tag="ot")
                        for co in range(c // 512):
                            pt = pso.tile([128, 512], FP, tag="pso")
                            for mb in range(MB):
                                nc.tensor.matmul(
                                    pt,
                                    lhsT=aoT_sb[:, mb, nb * 128:(nb + 1) * 128],
                                    rhs=wp_sb[:, mb, co * 512:(co + 1) * 512],
                                    start=(mb == 0),
                                    stop=(mb == MB - 1),
                                )
                            if co == 0:
                                nc.vector.tensor_copy(
                                    ot[:, co * 512:(co + 1) * 512], pt
                                )
                            else:
                                nc.scalar.copy(ot[:, co * 512:(co + 1) * 512], pt)
                        q_ = nc.sync if nb % 2 == 0 else nc.gpsimd
                        q_.dma_start(
                            out.rearrange("(nb p) co -> p nb co", p=128)[:, nb, :],
                            ot,
                        )
    nc.compile()
    return nc


def make_in_maps(x, mask, wq, wk, wv, wp):
    """Host-side sharding: per-core input dict (bf16)."""
    bf16 = ml_dtypes.bfloat16
    pselv = np.zeros((2, 128), dtype=np.float32)
    pselv[0, 0:64] = 1.0
    pselv[1, 64:128] = 1.0
    xTb = [np.ascontiguousarray(x[b].T).astype(bf16) for b in range(B)]
    mTb = [np.ascontiguousarray(mask[b].T).astype(bf16) for b in range(B)]
    in_maps = []
    for core in range(NCORES):
        b = core // 2
        g = core % 2
        cs = slice(g * 512, (g + 1) * 512)
        in_maps.append({
            "xT": xTb[b],
            "wqT": np.ascontiguousarray(wq[cs, :].T).astype(bf16),
            "wkT": np.ascontiguousarray(wk[cs, :].T).astype(bf16),
            "wvT": np.ascontiguousarray(wv[cs, :].T).astype(bf16),
            "wpT": np.ascontiguousarray(wp[:, cs].T).astype(bf16),
            "maskT": mTb[b],
            "psel": pselv,
        })
    return in_maps


_NC_CACHE = {}


def _get_nc():
    if "nc" not in _NC_CACHE:
        _NC_CACHE["nc"] = build_attention()
    return _NC_CACHE["nc"]


def kernel(x, mask, wq, wk, wv, wp, bp, _trace=False, _trace_kwargs=None):
    x = np.asarray(x, dtype=np.float32)
    mask = np.asarray(mask)
    wq = np.asarray(wq, dtype=np.float32)
    wk = np.asarray(wk, dtype=np.float32)
    wv = np.asarray(wv, dtype=np.float32)
    wp = np.asarray(wp, dtype=np.float32)
    bp = np.asarray(bp, dtype=np.float32)

    nc = _get_nc()
    in_maps = make_in_maps(x, mask, wq, wk, wv, wp)
    kw = {}
    if _trace:
        kw = {"trace": True, **(_trace_kwargs or {})}
    res = run_bass_kernel_spmd(nc, in_maps, list(range(NCORES)), **kw)
    outs = [np.asarray(r["out"], dtype=np.float32) for r in res.results]
    full = np.empty((B, N, DIM), dtype=np.float32)
    for b in range(B):
        full[b] = outs[2 * b] + outs[2 * b + 1] + bp[None, :]
    if _trace:
        return full, res
    return full


if __name__ == "__main__":
    nc = build_attention()
    print("built ok")


# revision 9
# speedup vs baseline: 1.4243x; 1.0145x over previous
"""Multi-head attention (B=4, N=2048, C=1024, H=16) on 8 trn2 NeuronCores.

Sharding: data-parallel over batch (4) x tensor-parallel over heads (2).
Core c handles batch c//2, heads [8*(c%2), 8*(c%2)+8). Each core computes a
partial output projection; the host sums core pairs and adds the bias.

v3: phase 2 is ScalarE-bound (256 exp ACTIVATEs of [128,1024]), so the
remaining QKV projection work is hidden under it:
- Unit order is head-block-OUTER ((hb, qh, h, kb)): q/k projections for
  blocks 1-3 (and v blocks 8-15) INJECT into phase 2's TensorE slack,
  accumulating in half-width slices of the rotating score PSUM tiles.
  Injected q/k groups re-stream their xT slice from DRAM (prefetched one
  injection ahead on the gpsimd queue) so xT need not stay in SBUF.
- Serial prologue: only q/k block 0 + v blocks 0..7.
- Input DMAs: critical loads first, the 8MB mask queued BEHIND them.
- Phase-1/prologue evacuations on ScalarE; injected evacuations on VectorE.
- exp table preloaded early; LOOK=2 score pipeline (3 rotating sc tiles).
- Denominator rows staged via dtmp->DMA on the gpsimd queue; epilogue
  emission delayed past the next unit's mask-mul.
- Tail: one reciprocal, K=2 pair-select broadcast matmuls, 4 wide in-place
  normalizations, projection (evacs split V/S), out DMAs on both queues.
"""

import os
import sys

for p in ("/opt/trn_rl_repo", "/root/.axon_site/_ro/trn_rl_repo"):
    if os.path.isdir(p) and p not in sys.path:
        sys.path.insert(0, p)

import ml_dtypes
import numpy as np

import concourse.bacc as bacc
import concourse.tile as tile
from concourse import mybir
from concourse.bass_utils import run_bass_kernel_spmd

FP = mybir.dt.float32
FR = mybir.dt.float32r
BF = mybir.dt.bfloat16
EXP = mybir.ActivationFunctionType.Exp

DIM = 1024
NUM_HEADS = 16
HEAD_DIM = 64
SCALE = HEAD_DIM ** -0.5
B, N = 4, 2048
NCORES = 8


def build_attention(n=N, c=DIM, cp=DIM // 2, hd=HEAD_DIM, scale=SCALE):
    """Emit the per-core program. All cores run the same code (SPMD)."""
    hpc = cp // hd          # heads on this core (8)
    CB = c // 128           # contraction blocks for QKV (8)
    MB = cp // 128          # c' blocks (4)
    NB = n // 128           # token/key blocks (16)
    QC = n // 512           # 512-col chunks (4)
    QW = 1024               # phase-2 q chunk width
    QH = n // QW            # phase-2 outer q chunks (2)
    hd1 = hd + 1            # v augmented with ones column
    PV = 8                  # v blocks computed in the serial prologue

    nc = bacc.Bacc("TRN2", target_bir_lowering=False, debug=False)

    xT = nc.dram_tensor("xT", [c, n], BF, kind="ExternalInput").ap()
    wqT = nc.dram_tensor("wqT", [c, cp], BF, kind="ExternalInput").ap()
    wkT = nc.dram_tensor("wkT", [c, cp], BF, kind="ExternalInput").ap()
    wvT = nc.dram_tensor("wvT", [c, cp], BF, kind="ExternalInput").ap()
    wpT = nc.dram_tensor("wpT", [cp, c], BF, kind="ExternalInput").ap()
    maskT = nc.dram_tensor("maskT", [n, n], BF, kind="ExternalInput").ap()
    psel = nc.dram_tensor("psel", [2, 128], BF, kind="ExternalInput").ap()
    out = nc.dram_tensor("out", [n, c], BF, kind="ExternalOutput").ap()

    xr = xT.rearrange("(cb p) n -> p cb n", p=128)
    mr = maskT.rearrange("(kb p) q -> p kb q", p=128)

    with tile.TileContext(nc) as tc:
        with (
            tc.tile_pool(name="persist", bufs=1) as pers,
            tc.tile_pool(name="d_pool", bufs=1) as dpool,
        ):
            qT_sb = pers.tile([128, MB, n], BF, tag="qT")
            kT_sb = pers.tile([128, MB, n], BF, tag="kT")
            vaug_sb = pers.tile([128, NB, hpc * hd1], BF, tag="vaug")
            mask_sb = pers.tile([128, NB, n], BF, tag="maskT")
            aoT_sb = pers.tile([128, MB, n], BF, tag="aoT")
            wp_sb = pers.tile([128, MB, c], BF, tag="wp")
            psel_sb = pers.tile([2, 128], BF, tag="psel")
            wq_sb = pers.tile([128, CB, cp], BF, tag="wqs")
            wk_sb = pers.tile([128, CB, cp], BF, tag="wks")
            wv_sb = pers.tile([128, CB, cp], BF, tag="wvs")
            d_sb = dpool.tile([hpc, n], FP, tag="dsum")

            def qk_mms(w_t, x_t, mb, dst_cols, pt, evac_engine):
                """q/k projection group; x_t columns already sliced."""
                dst, wn_t = dst_cols
                for cb in range(CB):
                    nc.tensor.matmul(
                        pt,
                        lhsT=w_t[:, cb, mb * 128:(mb + 1) * 128],
                        rhs=x_t[:, cb, :],
                        start=(cb == 0),
                        stop=(cb == CB - 1),
                    )
                evac_engine(dst, pt)

            def v_mms(x_t, wv_t, pt, nb, evac_engine):
                for cb in range(CB):
                    nc.tensor.matmul(
                        pt,
                        lhsT=x_t[:, cb, :],
                        rhs=wv_t[:, cb, :],
                        start=(cb == 0),
                        stop=(cb == CB - 1),
                    )
                dst3 = vaug_sb[:, nb, :].rearrange("p (h e) -> p h e", e=hd1)
                evac_engine(
                    dst3[:, :, 0:hd], pt.rearrange("p (h e) -> p h e", e=hd)
                )
                nc.vector.memset(dst3[:, :, hd:hd1], 1.0)

            # ---------------- Prologue ----------------
            with (
                tc.tile_pool(name="xt", bufs=1) as xpool,
                tc.tile_pool(name="warm", bufs=1) as warmp,
                tc.tile_pool(name="ps_qkv", bufs=4, space="PSUM") as pq,
            ):
                xT_sb = xpool.tile([128, CB, n], BF, tag="xT")

                # critical loads first; mask queued BEHIND them on sync.
                nc.sync.dma_start(wq_sb, wqT.rearrange("(cb p) m -> p cb m", p=128))
                for cb in range(0, CB, 2):
                    nc.sync.dma_start(xT_sb[:, cb, :], xr[:, cb, :])
                nc.gpsimd.dma_start(
                    wv_sb, wvT.rearrange("(cb p) m -> p cb m", p=128)
                )
                for cb in range(1, CB, 2):
                    nc.gpsimd.dma_start(xT_sb[:, cb, :], xr[:, cb, :])
                nc.sync.dma_start(wk_sb, wkT.rearrange("(cb p) m -> p cb m", p=128))
                nc.gpsimd.dma_start(psel_sb, psel)
                for kb in range(NB):
                    q_ = nc.sync if kb % 2 == 0 else nc.gpsimd
                    q_.dma_start(mask_sb[:, kb, :], mr[:, kb, :])
                nc.sync.dma_start(wp_sb, wpT.rearrange("(mb p) co -> p mb co", p=128))

                # preload the exp table set off the critical path
                wtile = warmp.tile([1, 16], FP, tag="warm")
                nc.vector.memset(wtile, 0.0)
                wout = warmp.tile([1, 16], BF, tag="warmo")
                nc.scalar.activation(wout, wtile, EXP, scale=1.0)

                for wn, w_t, dst in (("q", wq_sb, qT_sb), ("k", wk_sb, kT_sb)):
                    for qc in range(QC):
                        pt = pq.tile([128, 512], FP, tag="psqkv")
                        qk_mms(
                            w_t, xT_sb[:, :, qc * 512:(qc + 1) * 512], 0,
                            (dst[:, 0, qc * 512:(qc + 1) * 512], wn), pt,
                            nc.scalar.copy,
                        )
                for nb in range(PV):
                    pt = pq.tile([128, cp], FP, tag="psqkv")
                    v_mms(
                        xT_sb[:, :, nb * 128:(nb + 1) * 128], wv_sb, pt, nb,
                        nc.scalar.copy,
                    )

            # ---------------- Phase 2 with injected QKV ----------------
            # injections: v blocks PV..15 first (needed at unit kb), then
            # q/k pairs (mb 1..3) whose xT slice is re-streamed from DRAM.
            inject = [("v", nb) for nb in range(PV, NB)]
            for mb in range(1, MB):
                for qc in range(QC):
                    inject.append(("qk", mb, qc))  # q+k share the x slice

            with (
                tc.tile_pool(name="xq", bufs=2) as xqp,
                tc.tile_pool(name="ps_sc", bufs=3, space="PSUM") as psc,
                tc.tile_pool(name="ps_ao", bufs=1, space="PSUM") as pao,
                tc.tile_pool(name="s_exp", bufs=3) as sep,
                tc.tile_pool(name="s_m", bufs=4) as smp,
                tc.tile_pool(name="dtp", bufs=2) as dtp,
            ):
                # prefetch tiles for injections (rotating)
                inj_tiles = {}

                def prefetch_inject(i):
                    if i >= len(inject):
                        return
                    g = inject[i]
                    if g[0] == "v":
                        nb = g[1]
                        t = xqp.tile([128, CB, 512], BF, tag="xq", name="xq")
                        t = t[:, :, 0:128]
                        nc.gpsimd.dma_start(
                            t, xr[:, :, nb * 128:(nb + 1) * 128]
                        )
                    else:
                        qc = g[2]
                        t = xqp.tile([128, CB, 512], BF, tag="xq", name="xq")
                        nc.gpsimd.dma_start(
                            t, xr[:, :, qc * 512:(qc + 1) * 512]
                        )
                    inj_tiles[i] = t

                def emit_inject(i):
                    g = inject[i]
                    t = inj_tiles.pop(i)
                    if g[0] == "v":
                        gt = psc.tile([128, QW], FP, tag="sc", name="inj")
                        v_mms(t, wv_sb, gt[:, 0:cp], g[1], nc.vector.tensor_copy)
                    else:
                        mb, qc = g[1], g[2]
                        for wn, w_t, dst in (
                            ("q", wq_sb, qT_sb), ("k", wk_sb, kT_sb)
                        ):
                            gt = psc.tile([128, QW], FP, tag="sc", name="inj")
                            qk_mms(
                                w_t, t, mb,
                                (dst[:, mb, qc * 512:(qc + 1) * 512], wn),
                                gt[:, 0:512], nc.vector.tensor_copy,
                            )

                # injection points: v block nb at unit nb-PV (they are
                # needed at unit kb=nb); q/k pair mb at unit 12+16*(mb-1).
                inj_at = {}
                nqk = 0
                for i, g in enumerate(inject):
                    if g[0] == "v":
                        inj_at.setdefault(g[1] - PV, []).append(i)
                    else:
                        inj_at.setdefault(14 + 12 * nqk, []).append(i)
                        nqk += 1

                units = [
                    (hb, qh, 2 * hb + par, kb)
                    for hb in range(MB)
                    for qh in range(QH)
                    for par in range(2)
                    for kb in range(NB)
                ]
                LOOK = 2
                sc_map = {}
                ao_cur = [None]
                pending_epi = [None]
                prefetch_inject(0)
                prefetch_inject(1)
                next_pf = 2

                for idx in range(len(units) + LOOK):
                    if idx < len(units):
                        hb, qh, h, kb = units[idx]
                        qo = qh * QW
                        po = (h % 2) * hd
                        sc = psc.tile([128, QW], FP, tag="sc", name="sc")
                        sc_map[idx] = sc
                        for qs in range(QW // 512):
                            nc.tensor.matmul(
                                sc[:, qs * 512:(qs + 1) * 512],
                                lhsT=kT_sb[po:po + hd, hb, kb * 128:(kb + 1) * 128],
                                rhs=qT_sb[po:po + hd, hb, qo + qs * 512:qo + (qs + 1) * 512],
                                start=True,
                                stop=True,
                            )
                    j = idx - LOOK
                    if j < 0:
                        continue
                    hb, qh, h, kb = units[j]
                    qo = qh * QW
                    po = (h % 2) * hd
                    sc = sc_map.pop(j)
                    se = sep.tile([128, QW], BF, tag="se")
                    nc.scalar.activation(se, sc, EXP, scale=scale)
                    sm = smp.tile([128, QW], BF, tag="sm")
                    nc.vector.tensor_mul(sm, se, mask_sb[:, kb, qo:qo + QW])
                    if pending_epi[0] is not None:
                        pending_epi[0]()
                        pending_epi[0] = None
                    if kb == 0:
                        ao_cur[0] = pao.tile([hd1, QW], FP, tag="ao", name="ao")
                    ao = ao_cur[0]
                    for qs in range(QW // 512):
                        nc.tensor.matmul(
                            ao[:, qs * 512:(qs + 1) * 512],
                            lhsT=vaug_sb[:, kb, h * hd1:(h + 1) * hd1],
                            rhs=sm[:, qs * 512:(qs + 1) * 512],
                            start=(kb == 0),
                            stop=(kb == NB - 1),
                        )
                    for i in inj_at.get(j, ()):
                        emit_inject(i)
                        prefetch_inject(next_pf)
                        next_pf += 1
                    if kb == NB - 1:
                        def make_epi(h=h, po=po, hb=hb, qo=qo, ao=ao):
                            def epi():
                                nc.vector.tensor_copy(
                                    aoT_sb[po:po + hd, hb, qo:qo + QW],
                                    ao[0:hd, :],
                                )
                                dtmp = dtp.tile(
                                    [1, QW], FP, tag="dtmp", name="dt"
                                )
                                nc.vector.tensor_copy(dtmp, ao[hd:hd1, :])
                                nc.gpsimd.dma_start(
                                    d_sb[h:h + 1, qo:qo + QW], dtmp
                                )
                            return epi
                        pending_epi[0] = make_epi()
                if pending_epi[0] is not None:
                    pending_epi[0]()
                    pending_epi[0] = None

            # ---- tail: normalization broadcast + output projection ----
            with tc.tile_pool(name="dnp", bufs=1) as dnp:
                dinv = dnp.tile([hpc, n], FP, tag="dinv")
                nc.vector.reciprocal_approx_fast(dinv, d_sb)
                dinvb = dnp.tile([hpc, n], BF, tag="dinvb")
                nc.vector.tensor_copy(dinvb, dinv)
                with (
                    tc.tile_pool(name="d0p", bufs=2) as d0p,
                    tc.tile_pool(name="ps_bc", bufs=2, space="PSUM") as pbc,
                ):
                    for hb in range(MB):
                        d0 = d0p.tile([2, n], BF, tag="d0", name="d0")
                        nc.gpsimd.dma_start(
                            d0[0:1, :], dinvb[2 * hb:2 * hb + 1, :]
                        )
                        nc.gpsimd.dma_start(
                            d0[1:2, :], dinvb[2 * hb + 1:2 * hb + 2, :]
                        )
                        bc = pbc.tile([128, n], FP, tag="bc", name="bc")
                        for qc in range(QC):
                            cs_ = slice(qc * 512, (qc + 1) * 512)
                            nc.tensor.matmul(
                                bc[:, cs_], lhsT=psel_sb, rhs=d0[:, cs_],
                                start=True, stop=True,
                            )
                        nc.vector.tensor_mul(
                            aoT_sb[:, hb, :], aoT_sb[:, hb, :], bc
                        )

                with (
                    tc.tile_pool(name="ps_o", bufs=4, space="PSUM") as pso,
                    tc.tile_pool(name="osb", bufs=3) as osp,
                ):
                    for nb in range(NB):
                        ot = osp.tile([128, c], BF, tag="ot")
                        for co in range(c // 512):
                            pt = pso.tile([128, 512], FP, tag="pso")
                            for mb in range(MB):
                                nc.tensor.matmul(
                                    pt,
                                    lhsT=aoT_sb[:, mb, nb * 128:(nb + 1) * 128],
                                    rhs=wp_sb[:, mb, co * 512:(co + 1) * 512],
                                    start=(mb == 0),
                                    stop=(mb == MB - 1),
                                )
                            if co == 0:
                                nc.vector.tensor_copy(
                                    ot[:, co * 512:(co + 1) * 512], pt
                                )
                            else:
                                nc.scalar.copy(ot[:, co * 512:(co + 1) * 512], pt)
                        q_ = nc.sync if nb % 2 == 0 else nc.gpsimd
                        q_.dma_start(
                            out.rearrange("(nb p) co -> p nb co", p=128)[:, nb, :],
                            ot,
                        )
    nc.compile()
    return nc


def make_in_maps(x, mask, wq, wk, wv, wp):
    """Host-side sharding: per-core input dict (bf16)."""
    bf16 = ml_dtypes.bfloat16
    pselv = np.zeros((2, 128), dtype=np.float32)
    pselv[0, 0:64] = 1.0
    pselv[1, 64:128] = 1.0
    pselv = pselv.astype(bf16)
    xTb = [np.ascontiguousarray(x[b].T).astype(bf16) for b in range(B)]
    mTb = [np.ascontiguousarray(mask[b].T).astype(bf16) for b in range(B)]
    in_maps = []
    for core in range(NCORES):
        b = core // 2
        g = core % 2
        cs = slice(g * 512, (g + 1) * 512)
        in_maps.append({
            "xT": xTb[b],
            "wqT": np.ascontiguousarray(wq[cs, :].T).astype(bf16),
            "wkT": np.ascontiguousarray(wk[cs, :].T).astype(bf16),
            "wvT": np.ascontiguousarray(wv[cs, :].T).astype(bf16),
            "wpT": np.ascontiguousarray(wp[:, cs].T).astype(bf16),
            "maskT": mTb[b],
            "psel": pselv,
        })
    return in_maps


_NC_CACHE = {}


def _get_nc():
    if "nc" not in _NC_CACHE:
        _NC_CACHE["nc"] = build_attention()
    return _NC_CACHE["nc"]


def kernel(x, mask, wq, wk, wv, wp, bp, _trace=False, _trace_kwargs=None):
    x = np.asarray(x, dtype=np.float32)
    mask = np.asarray(mask)
    wq = np.asarray(wq, dtype=np.float32)
    wk = np.asarray(wk, dtype=np.float32)
    wv = np.asarray(wv, dtype=np.float32)
    wp = np.asarray(wp, dtype=np.float32)
    bp = np.asarray(bp, dtype=np.float32)

    nc = _get_nc()
    in_maps = make_in_maps(x, mask, wq, wk, wv, wp)
    kw = {}
    if _trace:
        kw = {"trace": True, **(_trace_kwargs or {})}
    res = run_bass_kernel_spmd(nc, in_maps, list(range(NCORES)), **kw)
    outs = [np.asarray(r["out"], dtype=np.float32) for r in res.results]
    full = np.empty((B, N, DIM), dtype=np.float32)
    for b in range(B):
        full[b] = outs[2 * b] + outs[2 * b + 1] + bp[None, :]
    if _trace:
        return full, res
    return full


if __name__ == "__main__":
    nc = build_attention()
    print("built ok")
